# revision 53
# baseline (speedup 1.0000x reference)
"""Locally-connected network (28x28 -> lc3x3 -> lc3x3 -> fc10) on 8 TRN2 cores.

The whole reference network is linear (two locally-connected layers + FC, no
activations), so on the host we fold it into a single affine map
    out[b, :] = x[b, :784] @ M + c          (M: [784, 10], c: [10])
computed in float64. The device kernel is a pure data-parallel, memory-bound
matmul over each core's 1024-sample shard; the stream of x bytes is the
bottleneck, so precision is allocated by contribution to the output:

  * Rows of M (pixels) are permuted by descending row energy ||M[k]||^2.
  * The top 4 k-tiles (448 rows, ~89% of output energy) stream in fp16.
  * The bottom 3 k-tiles (336 rows, ~11% of energy) stream in fp8e4m3.

Measured end-to-end relative error 1.2e-2 against the f64 reference, inside
the 2e-2 gate, with a ~30% smaller HBM stream than all-fp16.

Device layout (per core), one uint8 tensor (fp16/fp8 payloads bit-packed,
sliced out with AP bitcasts):
  xt[112, 256 + 11*1024]:
    bytes 0..255 per partition: weight block — 4 fp16 M k-tile blocks
      (10 cols each), bias c in fp16 at row 0 bytes 80..99, 3 fp8 M
      k-tile blocks at bytes 100..129.
    then batch slices, 11 bytes per sample per partition: 4 k-tiles x 2B
      fp16, then 3 k-tiles x 1B fp8, k-tile-major.
The weight block rides in slice 0's DMA so every matmul waits on at most
one DMA semaphore lane (TRN2 codegen rejects multi-wait consumers).

Per slice: 7 accumulating matmuls (4 fp16 + 3 fp8) plus a [1,10]x[1,SL]
ones-row matmul that folds the bias into PSUM (so no serialized ScalarE
bias-add chain), then a PSUM->SBUF fp16 downcast copy. The first NDVE
slices' copies run on DVE feeding one SP-ring store that launches while
later slices compute; the last slices' copies run on ScalarE feeding an
engine-ordered tail store. Decreasing slice sizes keep each slice's
post-DMA matmul+copy drain hidden under the next slice's transfer.
"""

import numpy as np
import ml_dtypes

import concourse.bass as bass
import concourse.tile as tile
from concourse import bacc, mybir
from concourse.bass_utils import run_bass_kernel_spmd

F8NP = ml_dtypes.float8_e4m3fn

N_CORES = 8
B = 8192
B_SHARD = B // N_CORES          # 1024
PIX = 784                       # 28*28
KP = 112                        # K-tile partition count; 784 = 7 * 112
NKT = PIX // KP                 # 7
NK16 = 4                        # fp16 k-tiles (highest-energy rows)
NK8 = NKT - NK16                # fp8 k-tiles
# Batch-slice sizes: small first slice so PE starts (and its clock ramps)
# early, moderate middle slices so each slice's matmul+copy drain hides
# under the next transfer, tiny last slice so the post-stream drain before
# the tail store is short.
SLICES = (192, 224, 320, 192, 96)
NSLICE = len(SLICES)
SOFF = tuple(sum(SLICES[:i]) for i in range(NSLICE + 1))  # batch offsets
NDVE = 4                        # slices copied on DVE (rest on ScalarE)
MWB = 256                       # weight block bytes per partition
SLB = 2 * NK16 + NK8            # 12 bytes per sample per partition
NB = MWB + SLB * B_SHARD
NOUT = 10
BIAS_B = NK16 * 2 * NOUT        # bias fp16 bytes, after the fp16 M blocks
M8_B = BIAS_B + 2 * NOUT        # fp8 M blocks


def _lc_dense(w, H, W_, oh, ow):
    """Dense [H*W_, oh*ow] matrix of one 3x3 locally-connected layer."""
    w = np.asarray(w, np.float64).reshape(oh, ow, 9)
    M = np.zeros((H * W_, oh * ow), np.float64)
    ox, oy = np.meshgrid(np.arange(oh), np.arange(ow), indexing="ij")
    col = (ox * ow + oy).ravel()
    for i in range(3):
        for j in range(3):
            row = ((ox + i) * W_ + (oy + j)).ravel()
            M[row, col] += w[:, :, i * 3 + j].ravel()
    return M


def _fold(w1, b1, w2, b2, fc_w, fc_b):
    W1 = _lc_dense(w1, 28, 28, 26, 26)          # [784, 676]
    W2 = _lc_dense(w2, 26, 26, 24, 24)          # [676, 576]
    fcw = np.asarray(fc_w, np.float64)          # [10, 576]
    M = W1 @ W2 @ fcw.T                         # [784, 10]
    c = (
        np.asarray(b1, np.float64).reshape(-1) @ W2
        + np.asarray(b2, np.float64).reshape(-1)
    ) @ fcw.T + np.asarray(fc_b, np.float64)    # [10]
    return M.astype(np.float32), c.astype(np.float32)


def _build_bass():
    nc = bacc.Bacc("TRN2", target_bir_lowering=False, debug=False)
    u8 = mybir.dt.uint8
    f16 = mybir.dt.float16
    f8 = mybir.dt.float8e4
    f32 = mybir.dt.float32
    xt = nc.declare_dram_parameter("xt", [KP, NB], u8, isOutput=False)
    out = nc.declare_dram_parameter("out", [NOUT, B_SHARD], f16, isOutput=True)

    with tile.TileContext(nc) as tc:
        with (
            tc.tile_pool(name="xp", bufs=NSLICE) as xp,
            tc.tile_pool(name="pp", bufs=NSLICE, space="PSUM") as pp,
            tc.tile_pool(name="wp", bufs=1, space="PSUM") as wp,
            tc.tile_pool(name="op", bufs=2) as op,
        ):
            # Ones row for the bias-fold matmul; DVE memset, no input deps.
            ones = op.tile([1, max(SLICES)], f16)
            nc.vector.memset(ones[:], 1.0)

            # Slice 0's DMA also carries the weight block.
            t0 = xp.tile([KP, MWB + SLB * SLICES[0]], u8)
            nc.sync.dma_start(t0[:], xt[:, 0 : MWB + SLB * SLICES[0]])

            def m16(kt):
                return t0[:, kt * 2 * NOUT : (kt + 1) * 2 * NOUT].bitcast(f16)

            def m8(j):
                return t0[:, M8_B + j * NOUT : M8_B + (j + 1) * NOUT].bitcast(f8)

            # Absorb the t0-DMA and ones-memset waits once on PE so real
            # matmuls wait on at most one semaphore lane each.
            warm = wp.tile([NOUT, 1], f32)
            nc.tensor.matmul(
                warm[:], m16(0), t0[:, 0:2].bitcast(f16), start=True, stop=True
            )
            warm2 = wp.tile([1, 1], f32)
            nc.tensor.matmul(
                warm2[:], ones[0:1, 0:1], ones[0:1, 0:1], start=True, stop=True
            )

            xs = [t0]
            for s in range(1, NSLICE):
                t = xp.tile([KP, SLB * SLICES[s]], u8)
                ring = nc.sync if s % 2 == 0 else nc.scalar
                ring.dma_start(
                    t[:], xt[:, MWB + SLB * SOFF[s] : MWB + SLB * SOFF[s + 1]]
                )
                xs.append(t)

            o = op.tile([NOUT, B_SHARD], f16)
            for s in range(NSLICE):
                base = MWB if s == 0 else 0
                sl = SLICES[s]
                ps_full = pp.tile([NOUT, max(SLICES)], f32)
                ps = ps_full[:, 0:sl]
                for kt in range(NK16):
                    nc.tensor.matmul(
                        ps[:],
                        m16(kt),
                        xs[s][:, base + kt * 2 * sl : base + (kt + 1) * 2 * sl]
                        .bitcast(f16),
                        start=(kt == 0),
                        stop=False,
                    )
                for j in range(NK8):
                    off = base + NK16 * 2 * sl + j * sl
                    nc.tensor.matmul(
                        ps[:],
                        m8(j),
                        xs[s][:, off : off + sl].bitcast(f8),
                        start=False,
                        stop=False,
                    )
                nc.tensor.matmul(
                    ps[:],
                    t0[0:1, BIAS_B : BIAS_B + 2 * NOUT].bitcast(f16),
                    ones[0:1, 0:sl],
                    start=False,
                    stop=True,
                )
                # PSUM->SBUF fp16 downcast: first half of the slices on DVE,
                # second half on ScalarE, so the late copies don't queue
                # behind the early ones and each store waits on one engine.
                dst = o[:, SOFF[s] : SOFF[s + 1]]
                if s < NDVE:
                    nc.vector.tensor_scalar_add(dst, ps[:], 0.0)
                else:
                    nc.scalar.copy(dst, ps[:])
                if s == NDVE - 1:
                    # Store for the DVE-copied slices (SP ring, waits DVE)
                    # launches while the later slices still compute.
                    nc.sync.dma_start(out[:, 0 : SOFF[s + 1]], o[:, 0 : SOFF[s + 1]])
            # Tail store rides the scalar ring right behind the final copy:
            # engine-ordered, no cross-engine wait.
            nc.scalar.dma_start(
                out[:, SOFF[NDVE] :], o[:, SOFF[NDVE] :]
            )
    nc.finalize()
    return nc


def _prepare(inputs):
    x = np.asarray(inputs["x"], np.float32)
    M, c = _fold(
        inputs["w1"], inputs["b1"], inputs["w2"], inputs["b2"],
        inputs["fc_w"], inputs["fc_b"],
    )
    # Permute pixel rows by descending output energy; low-energy tail rows
    # (k-tiles 5,6) carry ~4% of output energy and stream in fp8.
    perm = np.argsort(-(M.astype(np.float64) ** 2).sum(axis=1), kind="stable")
    Mp = M[perm]

    mw = np.zeros((KP, MWB), np.uint8)
    for kt in range(NK16):
        mw[:, kt * 2 * NOUT : (kt + 1) * 2 * NOUT] = (
            Mp[kt * KP : (kt + 1) * KP].astype(np.float16).view(np.uint8)
        )
    mw[0, BIAS_B : BIAS_B + 2 * NOUT] = c.astype(np.float16).view(np.uint8)
    for j in range(NK8):
        mw[:, M8_B + j * NOUT : M8_B + (j + 1) * NOUT] = (
            Mp[(NK16 + j) * KP : (NK16 + j + 1) * KP].astype(F8NP).view(np.uint8)
        )

    xr = x.reshape(B, PIX)[:, perm]
    x16 = xr[:, : NK16 * KP].astype(np.float16)     # [B, 560]
    x8 = xr[:, NK16 * KP :].astype(F8NP)            # [B, 224]

    in_maps = []
    for i in range(N_CORES):
        lo, hi = i * B_SHARD, (i + 1) * B_SHARD
        arr = np.empty((KP, NB), np.uint8)
        arr[:, 0:MWB] = mw
        for s in range(NSLICE):
            sl = SLICES[s]
            bs, be = lo + SOFF[s], lo + SOFF[s + 1]
            blk16 = (
                x16[bs:be]
                .reshape(sl, NK16, KP)
                .transpose(2, 1, 0)                  # [KP, NK16, sl] f16
                .copy()
                .view(np.uint8)
                .reshape(KP, NK16 * 2 * sl)
            )
            blk8 = (
                x8[bs:be]
                .reshape(sl, NK8, KP)
                .transpose(2, 1, 0)                  # [KP, NK8, sl] f8
                .copy()
                .view(np.uint8)
                .reshape(KP, NK8 * sl)
            )
            col = MWB + SLB * SOFF[s]
            arr[:, col : col + NK16 * 2 * sl] = blk16
            arr[:, col + NK16 * 2 * sl : col + SLB * sl] = blk8
        in_maps.append({"xt": arr})
    return in_maps


def _build_for_sim(inputs):
    return _build_bass(), _prepare(inputs)[0]


def _run(inputs, trace=False, trace_cores=None):
    in_maps = _prepare(inputs)
    nc = _build_bass()
    res = run_bass_kernel_spmd(
        nc,
        in_maps,
        list(range(N_CORES)),
        trace=trace,
        trace_cores=trace_cores,
    )
    out = np.concatenate(
        [np.asarray(res.results[i]["out"]).T for i in range(N_CORES)], axis=0
    ).astype(np.float32)
    return out, res


def kernel(**inputs) -> np.ndarray:
    out, _ = _run(inputs, trace=False)
    return out


# revision 54
# speedup vs baseline: 1.0050x; 1.0050x over previous
"""Locally-connected network (28x28 -> lc3x3 -> lc3x3 -> fc10) on 8 TRN2 cores.

The whole reference network is linear (two locally-connected layers + FC, no
activations), so on the host we fold it into a single affine map
    out[b, :] = x[b, :784] @ M + c          (M: [784, 10], c: [10])
computed in float64. The device kernel is a pure data-parallel, memory-bound
matmul over each core's 1024-sample shard; the stream of x bytes is the
bottleneck, so precision is allocated by contribution to the output:

  * Rows of M (pixels) are permuted by descending row energy ||M[k]||^2.
  * The top 4 k-tiles (448 rows, ~89% of output energy) stream in fp16.
  * The bottom 3 k-tiles (336 rows, ~11% of energy) stream in fp8e4m3.

Measured end-to-end relative error 1.2e-2 against the f64 reference, inside
the 2e-2 gate, with a ~30% smaller HBM stream than all-fp16.

Device layout (per core), one uint8 tensor (fp16/fp8 payloads bit-packed,
sliced out with AP bitcasts):
  xt[112, 256 + 11*1024]:
    bytes 0..255 per partition: weight block — 4 fp16 M k-tile blocks
      (10 cols each), bias c in fp16 at row 0 bytes 80..99, 3 fp8 M
      k-tile blocks at bytes 100..129.
    then batch slices, 11 bytes per sample per partition: 4 k-tiles x 2B
      fp16, then 3 k-tiles x 1B fp8, k-tile-major.
The weight block rides in slice 0's DMA so every matmul waits on at most
one DMA semaphore lane (TRN2 codegen rejects multi-wait consumers).

Per slice: 7 accumulating matmuls (4 fp16 + 3 fp8) plus a [1,10]x[1,SL]
ones-row matmul that folds the bias into PSUM (so no serialized ScalarE
bias-add chain), then a PSUM->SBUF fp16 downcast copy. The first NDVE
slices' copies run on DVE feeding one SP-ring store that launches while
later slices compute; the last slices' copies run on ScalarE feeding an
engine-ordered tail store. Decreasing slice sizes keep each slice's
post-DMA matmul+copy drain hidden under the next slice's transfer.
"""

import numpy as np
import ml_dtypes

import concourse.bass as bass
import concourse.tile as tile
from concourse import bacc, mybir
from concourse.bass_utils import run_bass_kernel_spmd

F8NP = ml_dtypes.float8_e4m3fn

N_CORES = 8
B = 8192
B_SHARD = B // N_CORES          # 1024
PIX = 784                       # 28*28
KP = 112                        # K-tile partition count; 784 = 7 * 112
NKT = PIX // KP                 # 7
NK16 = 4                        # fp16 k-tiles (highest-energy rows)
NK8 = NKT - NK16                # fp8 k-tiles
# Batch-slice sizes: small first slice so PE starts (and its clock ramps)
# early, moderate middle slices so each slice's matmul+copy drain hides
# under the next transfer, tiny last slice so the post-stream drain before
# the tail store is short.
SLICES = (192, 224, 320, 192, 96)
NSLICE = len(SLICES)
SOFF = tuple(sum(SLICES[:i]) for i in range(NSLICE + 1))  # batch offsets
NDVE = 4                        # slices copied on DVE (rest on ScalarE)
MWB = 132                       # weight block bytes per partition (130 used, even-aligned)
SLB = 2 * NK16 + NK8            # 12 bytes per sample per partition
NB = MWB + SLB * B_SHARD
NOUT = 10
BIAS_B = NK16 * 2 * NOUT        # bias fp16 bytes, after the fp16 M blocks
M8_B = BIAS_B + 2 * NOUT        # fp8 M blocks


def _lc_dense(w, H, W_, oh, ow):
    """Dense [H*W_, oh*ow] matrix of one 3x3 locally-connected layer."""
    w = np.asarray(w, np.float64).reshape(oh, ow, 9)
    M = np.zeros((H * W_, oh * ow), np.float64)
    ox, oy = np.meshgrid(np.arange(oh), np.arange(ow), indexing="ij")
    col = (ox * ow + oy).ravel()
    for i in range(3):
        for j in range(3):
            row = ((ox + i) * W_ + (oy + j)).ravel()
            M[row, col] += w[:, :, i * 3 + j].ravel()
    return M


def _fold(w1, b1, w2, b2, fc_w, fc_b):
    W1 = _lc_dense(w1, 28, 28, 26, 26)          # [784, 676]
    W2 = _lc_dense(w2, 26, 26, 24, 24)          # [676, 576]
    fcw = np.asarray(fc_w, np.float64)          # [10, 576]
    M = W1 @ W2 @ fcw.T                         # [784, 10]
    c = (
        np.asarray(b1, np.float64).reshape(-1) @ W2
        + np.asarray(b2, np.float64).reshape(-1)
    ) @ fcw.T + np.asarray(fc_b, np.float64)    # [10]
    return M.astype(np.float32), c.astype(np.float32)


def _build_bass():
    nc = bacc.Bacc("TRN2", target_bir_lowering=False, debug=False)
    u8 = mybir.dt.uint8
    f16 = mybir.dt.float16
    f8 = mybir.dt.float8e4
    f32 = mybir.dt.float32
    xt = nc.declare_dram_parameter("xt", [KP, NB], u8, isOutput=False)
    out = nc.declare_dram_parameter("out", [NOUT, B_SHARD], f16, isOutput=True)

    with tile.TileContext(nc) as tc:
        with (
            tc.tile_pool(name="xp", bufs=NSLICE) as xp,
            tc.tile_pool(name="pp", bufs=NSLICE, space="PSUM") as pp,
            tc.tile_pool(name="wp", bufs=1, space="PSUM") as wp,
            tc.tile_pool(name="op", bufs=2) as op,
        ):
            # Ones row for the bias-fold matmul; DVE memset, no input deps.
            ones = op.tile([1, max(SLICES)], f16)
            nc.vector.memset(ones[:], 1.0)

            # Slice 0's DMA also carries the weight block.
            t0 = xp.tile([KP, MWB + SLB * SLICES[0]], u8)
            nc.sync.dma_start(t0[:], xt[:, 0 : MWB + SLB * SLICES[0]])

            def m16(kt):
                return t0[:, kt * 2 * NOUT : (kt + 1) * 2 * NOUT].bitcast(f16)

            def m8(j):
                return t0[:, M8_B + j * NOUT : M8_B + (j + 1) * NOUT].bitcast(f8)

            # Absorb the t0-DMA and ones-memset waits once on PE so real
            # matmuls wait on at most one semaphore lane each.
            warm = wp.tile([NOUT, 1], f32)
            nc.tensor.matmul(
                warm[:], m16(0), t0[:, 0:2].bitcast(f16), start=True, stop=True
            )
            warm2 = wp.tile([1, 1], f32)
            nc.tensor.matmul(
                warm2[:], ones[0:1, 0:1], ones[0:1, 0:1], start=True, stop=True
            )

            xs = [t0]
            for s in range(1, NSLICE):
                t = xp.tile([KP, SLB * SLICES[s]], u8)
                ring = nc.sync if s % 2 == 0 else nc.scalar
                ring.dma_start(
                    t[:], xt[:, MWB + SLB * SOFF[s] : MWB + SLB * SOFF[s + 1]]
                )
                xs.append(t)

            o = op.tile([NOUT, B_SHARD], f16)
            for s in range(NSLICE):
                base = MWB if s == 0 else 0
                sl = SLICES[s]
                ps_full = pp.tile([NOUT, max(SLICES)], f32)
                ps = ps_full[:, 0:sl]
                for kt in range(NK16):
                    nc.tensor.matmul(
                        ps[:],
                        m16(kt),
                        xs[s][:, base + kt * 2 * sl : base + (kt + 1) * 2 * sl]
                        .bitcast(f16),
                        start=(kt == 0),
                        stop=False,
                    )
                for j in range(NK8):
                    off = base + NK16 * 2 * sl + j * sl
                    nc.tensor.matmul(
                        ps[:],
                        m8(j),
                        xs[s][:, off : off + sl].bitcast(f8),
                        start=False,
                        stop=False,
                    )
                nc.tensor.matmul(
                    ps[:],
                    t0[0:1, BIAS_B : BIAS_B + 2 * NOUT].bitcast(f16),
                    ones[0:1, 0:sl],
                    start=False,
                    stop=True,
                )
                # PSUM->SBUF fp16 downcast: first half of the slices on DVE,
                # second half on ScalarE, so the late copies don't queue
                # behind the early ones and each store waits on one engine.
                dst = o[:, SOFF[s] : SOFF[s + 1]]
                if s < NDVE:
                    nc.vector.tensor_scalar_add(dst, ps[:], 0.0)
                else:
                    nc.scalar.copy(dst, ps[:])
                if s == NDVE - 1:
                    # Store for the DVE-copied slices (SP ring, waits DVE)
                    # launches while the later slices still compute.
                    nc.sync.dma_start(out[:, 0 : SOFF[s + 1]], o[:, 0 : SOFF[s + 1]])
            # Tail store rides the scalar ring right behind the final copy:
            # engine-ordered, no cross-engine wait.
            nc.scalar.dma_start(
                out[:, SOFF[NDVE] :], o[:, SOFF[NDVE] :]
            )
    nc.finalize()
    return nc


def _prepare(inputs):
    x = np.asarray(inputs["x"], np.float32)
    M, c = _fold(
        inputs["w1"], inputs["b1"], inputs["w2"], inputs["b2"],
        inputs["fc_w"], inputs["fc_b"],
    )
    # Permute pixel rows by descending output energy; low-energy tail rows
    # (k-tiles 5,6) carry ~4% of output energy and stream in fp8.
    perm = np.argsort(-(M.astype(np.float64) ** 2).sum(axis=1), kind="stable")
    Mp = M[perm]

    mw = np.zeros((KP, MWB), np.uint8)
    for kt in range(NK16):
        mw[:, kt * 2 * NOUT : (kt + 1) * 2 * NOUT] = (
            Mp[kt * KP : (kt + 1) * KP].astype(np.float16).view(np.uint8)
        )
    mw[0, BIAS_B : BIAS_B + 2 * NOUT] = c.astype(np.float16).view(np.uint8)
    for j in range(NK8):
        mw[:, M8_B + j * NOUT : M8_B + (j + 1) * NOUT] = (
            Mp[(NK16 + j) * KP : (NK16 + j + 1) * KP].astype(F8NP).view(np.uint8)
        )

    xr = x.reshape(B, PIX)[:, perm]
    x16 = xr[:, : NK16 * KP].astype(np.float16)     # [B, 560]
    x8 = xr[:, NK16 * KP :].astype(F8NP)            # [B, 224]

    in_maps = []
    for i in range(N_CORES):
        lo, hi = i * B_SHARD, (i + 1) * B_SHARD
        arr = np.empty((KP, NB), np.uint8)
        arr[:, 0:MWB] = mw
        for s in range(NSLICE):
            sl = SLICES[s]
            bs, be = lo + SOFF[s], lo + SOFF[s + 1]
            blk16 = (
                x16[bs:be]
                .reshape(sl, NK16, KP)
                .transpose(2, 1, 0)                  # [KP, NK16, sl] f16
                .copy()
                .view(np.uint8)
                .reshape(KP, NK16 * 2 * sl)
            )
            blk8 = (
                x8[bs:be]
                .reshape(sl, NK8, KP)
                .transpose(2, 1, 0)                  # [KP, NK8, sl] f8
                .copy()
                .view(np.uint8)
                .reshape(KP, NK8 * sl)
            )
            col = MWB + SLB * SOFF[s]
            arr[:, col : col + NK16 * 2 * sl] = blk16
            arr[:, col + NK16 * 2 * sl : col + SLB * sl] = blk8
        in_maps.append({"xt": arr})
    return in_maps


def _build_for_sim(inputs):
    return _build_bass(), _prepare(inputs)[0]


def _run(inputs, trace=False, trace_cores=None):
    in_maps = _prepare(inputs)
    nc = _build_bass()
    res = run_bass_kernel_spmd(
        nc,
        in_maps,
        list(range(N_CORES)),
        trace=trace,
        trace_cores=trace_cores,
    )
    out = np.concatenate(
        [np.asarray(res.results[i]["out"]).T for i in range(N_CORES)], axis=0
    ).astype(np.float32)
    return out, res


def kernel(**inputs) -> np.ndarray:
    out, _ = _run(inputs, trace=False)
    return out
